# revision 30
# baseline (speedup 1.0000x reference)
"""Trainium2 Bass kernel for nn_Decoding_Layer (dense transformer decoder layer).

Sharding: 8 cores = 4 batches x 2 sequence-halves. Each core computes one
512-row query block of one batch end-to-end (no collectives). K/V projections
are computed over the full 1024-key sequence per core; causal masking is
data-driven (host-fed mask slice for the diagonal 512x512 block plus a V-row
mask that zeroes invalid key blocks), so all 8 cores run one uniform program.

All big matmuls run as float32r (fp32 operands truncated to ~fp22 inside the
PE at full bf16-rate) with fp32 PSUM accumulation. Activations are kept
feature-major ("transposed", [feat, row]) so weights load untransposed and
per-feature biases fold into per-partition bias slots during PSUM drains.

v3 pipeline structure:
- Attention slots are software-pipelined with a skew of 2 key-chunks: the AV
  matmuls for chunk kc issue only after the scores for kc+2, so the ACT-engine
  exp (the only ACT work inside attention slots) has ~2.3us of PE cover and
  the PE never stalls on the scores->exp->AV chain.
- All PSUM drains inside attention phases run on the Pool engine (gpsimd) as
  fused (psum+bias)[+residual] ops; ACT does exp exclusively there.
- Self-attention exploits causality: key chunks are iterated diagonal-first
  (so PSUM accumulation windows start full-width), and scores/exp/AV for
  diagonal chunk c only cover the valid query window [128c, 512). The mask is
  applied only to the 128-wide boundary triangle strip.
- Softmax denominators ride as a 65th V row; their reciprocals use the
  single-instruction DVE reciprocal_approx_fast (~5x cheaper than
  reciprocal()).
- q2 projections interleave into the self-attention kc loops (1 matmul per
  chunk slot), k2 rides as slot-end bursts (lagged one slot), wo1 interleaves
  into cross-attention slots 0-3, and LN1 runs as a burst at cross slot 3 so
  its vector work hides under slots 4-7.
- LN2/LN3 statistics (mean/var partial-sum matmuls + squares) fold into the
  wo2/ffn2 drain streams so only a short normalize tail remains.
- Weight streams split across two DMA queues per FFN gemm so the fp32 weight
  traffic (16 MiB per gemm) never falls behind the PE.
"""

import sys

if "/opt/trn_rl_repo" not in sys.path:
    sys.path.insert(0, "/opt/trn_rl_repo")

import numpy as np

import concourse.bass as bass
import concourse.mybir as mybir
import concourse.tile as tile
from concourse import bass_utils
from concourse.bass_utils import run_bass_kernel_spmd

# walrus ships with --enable-ldw-opt=false; enabling it lets codegen overlap
# the per-matmul 4-byte weight loads, which otherwise serialize with the
# matmul stream on this fp32r-heavy kernel.
_orig_run_command = bass_utils.run_command

def _patched_run_command(argv, **kw):
    argv = ["--enable-ldw-opt=true" if a == "--enable-ldw-opt=false" else a
            for a in argv]
    return _orig_run_command(argv, **kw)

bass_utils.run_command = _patched_run_command

f32 = mybir.dt.float32
f32r = mybir.dt.float32r
AF = mybir.ActivationFunctionType
ALU = mybir.AluOpType

B, S, D, H, DFF = 4, 1024, 1024, 16, 4096
DEPTH = D // H
R = 512          # rows (query block) per core
EPS = 1e-6
N_CORES = 8


def _split_waits(nc, maxw=1):
    """Walrus in this toolchain encodes at most one semaphore wait per
    instruction; Tile emits several. Move excess waits onto same-engine NOPs
    placed immediately before the instruction (sequential per-engine streams
    make this equivalent)."""
    for f in nc.m.functions:
        for bb in f.blocks:
            out = []
            for inst in bb.instructions:
                si = inst.sync_info
                if si is not None and len(si.on_wait) > maxw:
                    waits = list(si.on_wait)
                    keep, excess = waits[-maxw:], waits[:-maxw]
                    eng = getattr(inst, "engine", None)
                    k = 0
                    while excess:
                        chunk, excess = excess[:maxw], excess[maxw:]
                        out.append(mybir.InstNoOp(
                            name=f"{inst.name}_wsp{k}",
                            engine=eng,
                            bass_nofuse=True,
                            sync_info=mybir.SyncInfo(on_wait=chunk, on_update=[]),
                        ))
                        k += 1
                    inst.sync_info = mybir.SyncInfo(
                        on_wait=keep, on_update=list(si.on_update))
                out.append(inst)
            bb.instructions = out


def build_program():
    nc = bass.Bass("TRN2", target_bir_lowering=False, debug=False)

    def din(name, shape):
        return nc.dram_tensor(name, shape, f32, kind="ExternalInput").ap()

    dc_own_d = din("dc_own", [D, R])        # dec_input own rows, transposed
    dke_d = din("dke", [D, S])              # dec keys (reordered: ctx|diag), transposed
    enc_d = din("encT", [D, S])             # enc_output, transposed
    mask_d = din("maskT", [R, R])           # causal diag block, [key, q], pre * -8e9
    padb_d = din("padb", [128, 8])          # -1e9 * padding_mask, chunked
    vm1_d = din("vm1", [128, 8])            # self V-row mask (chunked)
    vm1r_d = din("vm1r", [128, 8, 16])      # same, replicated per head
    vm2_d = din("vm2", [128, 8])            # ones
    vm2r_d = din("vm2r", [128, 8, 16])      # ones
    ones_d = din("onesd", [128, 128])       # ones
    esel_d = din("esel", [128, 256])        # denom-broadcast selector
    w_d = {k: din(k, [D, D]) for k in ("wq1", "wk1", "wq2", "wk2", "wo1", "wo2")}
    fw1_d = din("fw1", [D, DFF])
    fw2_d = din("fw2", [DFF, D])
    bc_d = {k: din(k, [128, 8]) for k in
            ("bq1c", "bk1c", "bq2c", "bk2c", "bo1c", "bo2c", "fb2c",
             "g1c", "b1c", "g2c", "b2c", "g3c", "b3c")}
    fb1c_d = din("fb1c", [128, 32])
    out_d = nc.dram_tensor("outT", [D, R], f32, kind="ExternalOutput").ap()

    with tile.TileContext(nc) as tc:
        with tc.tile_pool(name="persist", bufs=1) as pp, \
             tc.tile_pool(name="consts", bufs=1) as cp:
            # ---- persistent SBUF ----
            arena = pp.tile([128, 16384], f32r, name="arena")     # 8 MiB
            dke = arena[:, 0:8192].rearrange("p (f r) -> p f r", f=8)
            encv = arena[:, 0:8192].rearrange("p (f r) -> p f r", f=8)
            kTv = arena[:, 8192:16384].rearrange("p (f r) -> p f r", f=8)
            vaug = pp.tile([128, 8, 16, 65], f32r, name="vaug")
            dc_own = pp.tile([128, 8, R], f32r, name="dc_own")
            qbuf = pp.tile([128, 8, R], f32r, name="qbuf")        # q1T -> q2T -> attn2T -> outT
            abufA = pp.tile([128, 8, R], f32r, name="abufA")      # attn1T -> x2pre/x2T
            xa = pp.tile([128, 8, R], f32r, name="xa")            # x1pre/x1T -> x3pre

            # ---- constants ----
            onesb = cp.tile([128, 128], f32r, name="onesb")
            esel = cp.tile([128, 256], f32r, name="esel")
            padb = cp.tile([128, 8], f32, name="padb")
            vm1 = cp.tile([128, 8], f32, name="vm1")
            vm2 = cp.tile([128, 8], f32, name="vm2")
            bcs = {k: cp.tile([128, 8], f32, name=k) for k in bc_d}
            fb1c = cp.tile([128, 32], f32, name="fb1c")

            # critical-path loads first: warmup needs only onesb; q1's weight
            # stream follows dc_own chunk 0 + bq1c on the sync queue so its
            # first chunk lands ~2us in. dc_own chunks 1-7 and the other
            # consts ride the GpSimd SWDGE queues; dke rides scalar.
            nc.sync.dma_start(out=onesb, in_=ones_d.bitcast(f32r))
            dco_r = dc_own_d.rearrange("(f p) r -> p f r", p=128).bitcast(f32r)
            nc.sync.dma_start(out=dc_own[:, 0, :], in_=dco_r[:, 0, :])
            nc.sync.dma_start(out=bcs["bq1c"], in_=bc_d["bq1c"])
            for kc in range(1, 8):
                nc.gpsimd.dma_start(out=dc_own[:, kc, :], in_=dco_r[:, kc, :])
            dke_r = dke_d.rearrange("(f p) r -> p f r", p=128).bitcast(f32r)
            for kc in range(8):
                nc.scalar.dma_start(out=dke[:, kc, :], in_=dke_r[:, kc, :])
            nc.gpsimd.dma_start(out=padb, in_=padb_d)
            nc.gpsimd.dma_start(out=vm1, in_=vm1_d)
            nc.gpsimd.dma_start(out=vm2, in_=vm2_d)
            for k in bcs:
                if k != "bq1c":
                    nc.gpsimd.dma_start(out=bcs[k], in_=bc_d[k])
            nc.gpsimd.dma_start(out=fb1c, in_=fb1c_d)
            nc.gpsimd.dma_start(out=esel, in_=esel_d.bitcast(f32r))

            ones1 = onesb[0:1, :]      # [1, 128] f32r
            onesp = onesb[:, 0:1]      # [128, 1] f32r

            # one kernel-lifetime weight-stream pool: per-phase pools would
            # re-allocate SBUF at each phase boundary, stalling the prefetch.
            wall = ctx_wp = tc.tile_pool(name="wall", bufs=6)
            wall = wall.__enter__()

            # masks live in a pool scoped to phases 1-2 only (its 8 KiB of
            # column space is needed for LN scratch later); the DMA still
            # issues near program start on the gpsimd queue.
            mk_ctx = tc.tile_pool(name="mk", bufs=1)
            mkp = mk_ctx.__enter__()
            masks = mkp.tile([128, 4, R], f32, name="masks")
            for c in range(4):
                nc.gpsimd.dma_start(out=masks[:, c, :],
                                    in_=mask_d[c * 128:(c + 1) * 128, :])

            # ~9us of dummy matmuls (input-independent: rhs re-reads onesb via
            # a 0-stride AP) pulls the PE HAM clock-gate to 8/8 before the
            # real stream begins; only the 64 KiB onesb DMA gates the start.
            warm_rhs = bass.AP(tensor=onesb.tensor, offset=onesb[:, :].offset,
                               ap=[list(onesb.ap[0]), [0, 4], [1, 128]])
            with tc.tile_pool(name="warm", bufs=1, space="PSUM") as wps:
                wtile = wps.tile([128, 512], f32, name="warm")
                for wi in range(24):
                    nc.tensor.matmul(
                        wtile[:],
                        lhsT=onesb[:, 0:128],
                        rhs=warm_rhs,
                        start=(wi == 0), stop=(wi == 23))

            # ---- helpers ----
            def gemm_TN(Wd, xt, KCn, MCn, NN, drain, wp, ps, dma=None,
                        dma2=None):
                """OUT^T[m-chunk, n] = sum_kc W[kc, m]^T @ xt(kc, n).
                xt(kc, n) -> [128, 512] f32r AP. drain(mi, n, psum_ap).
                dma2 (if set) takes odd-kc weight loads for queue balance."""
                dma = dma or nc.sync.dma_start
                g = max(1, 4 // NN)
                for mg in range(0, MCn, g):
                    gs = min(g, MCn - mg)
                    pps = {}
                    for i in range(gs):
                        for n in range(NN):
                            pps[(i, n)] = ps.tile([128, 512], f32, name="pp")
                    for kc in range(KCn):
                        wt = wp.tile([128, gs * 128], f32r, name="wt")
                        dq = dma2 if (dma2 is not None and kc % 2 == 1) else dma
                        dq(out=wt,
                           in_=Wd[kc * 128:(kc + 1) * 128,
                                  mg * 128:(mg + gs) * 128].bitcast(f32r))
                        for i in range(gs):
                            for n in range(NN):
                                nc.tensor.matmul(
                                    pps[(i, n)][:],
                                    lhsT=wt[:, i * 128:(i + 1) * 128],
                                    rhs=xt(kc, n),
                                    start=(kc == 0), stop=(kc == KCn - 1))
                    for i in range(gs):
                        for n in range(NN):
                            drain(mg + i, n, pps[(i, n)])

            def gemm_NT(Wd, xt_sb, KCn, RCn, NFn, drain, wp, ps, G=4,
                        dma=None, dma2=None):
                """OUT[r-chunk] = X @ W : lhsT = xt chunks, rhs = W cols.
                drain(rc, nf, psum_ap). xt_sb [128, KCn, S] f32r."""
                dma = dma or nc.scalar.dma_start
                for nf in range(NFn):
                    for rg in range(0, RCn, G):
                        gs = min(G, RCn - rg)
                        pps = [ps.tile([128, 512], f32, name="pp") for _ in range(gs)]
                        for kc in range(KCn):
                            wt = wp.tile([128, 512], f32r, name="wt")
                            dq = dma2 if (dma2 is not None and kc % 2 == 1) else dma
                            dq(out=wt,
                               in_=Wd[kc * 128:(kc + 1) * 128,
                                      nf * 512:(nf + 1) * 512].bitcast(f32r))
                            for i in range(gs):
                                nc.tensor.matmul(
                                    pps[i][:],
                                    lhsT=xt_sb[:, kc, (rg + i) * 128:(rg + i + 1) * 128],
                                    rhs=wt[:],
                                    start=(kc == 0), stop=(kc == KCn - 1))
                        for i in range(gs):
                            drain(rg + i, nf, pps[i])

            def attn_slot(f, q_sb, kT_sb, v_sb, is_self, epool,
                          ps_s, ps_av, fill_j, fill_end):
                """One head-pair attention slot with skew-2 pipelining:
                AV(kc) issues two chunk-slots after scores(kc), giving the
                ACT exp ~2.3us of PE cover. Self-attention iterates the
                diagonal key block first (full-width PSUM init) and windows
                scores/exp/AV on diagonal chunks to the valid query range.
                Returns the two [65,512] PSUM accumulators (row 64 = denom)."""
                avs = [ps_av.tile([65, 512], f32, name="av") for _ in range(2)]
                order = [4, 5, 6, 7, 0, 1, 2, 3] if is_self else list(range(8))

                def emit_av(kc, w, e):
                    for a in range(2):
                        nc.tensor.matmul(
                            avs[a][:, w:512],
                            lhsT=v_sb[:, kc, 2 * f + a, :],
                            rhs=e[:, a * 512 + w:(a + 1) * 512],
                            start=(kc == order[0]), stop=(kc == order[-1]))

                pend = []
                for j, kc in enumerate(order):
                    w = 128 * (kc - 4) if (is_self and kc >= 4) else 0
                    ss = ps_s.tile([128, 1024], f32, name="ss")
                    for a in range(2):
                        nc.tensor.matmul(
                            ss[:, a * 512 + w:(a + 1) * 512],
                            lhsT=kT_sb[64 * a:64 * (a + 1), f, kc * 128:(kc + 1) * 128],
                            rhs=q_sb[64 * a:64 * (a + 1), f, w:512],
                            start=True, stop=True)
                    if is_self and kc >= 4:
                        # boundary triangle strip only: queries [w, w+128)
                        c = kc - 4
                        strip = bass.AP(
                            tensor=ss.tensor, offset=ss[:, w:512].offset,
                            ap=[list(ss.ap[0]), [512, 2], [1, 128]])
                        mstrip = bass.AP(
                            tensor=masks.tensor,
                            offset=masks[:, c, w:512].offset,
                            ap=[list(masks.ap[0]), [0, 2], [1, 128]])
                        nc.vector.tensor_tensor(out=strip, in0=strip,
                                                in1=mstrip, op=ALU.add)
                    e = epool.tile([128, 1024], f32r, name="ee")
                    ewin = 512 - w
                    ss3 = bass.AP(tensor=ss.tensor, offset=ss[:, w:512].offset,
                                  ap=[list(ss.ap[0]), [512, 2], [1, ewin]])
                    e3 = bass.AP(tensor=e.tensor, offset=e[:, w:512].offset,
                                 ap=[list(e.ap[0]), [512, 2], [1, ewin]])
                    bias = 0.0 if is_self else padb[:, kc:kc + 1]
                    nc.scalar.activation(e3, ss3, AF.Exp, bias=bias, scale=0.125)
                    if len(pend) == 2:
                        emit_av(*pend.pop(0))
                    fill_j(j)
                    pend.append((kc, w, e))
                emit_av(*pend.pop(0))
                fill_end()
                emit_av(*pend.pop(0))
                return avs

            def attn_stash(f, avs, den, dst_fn):
                """Copy unnormalized numerators out of PSUM and stash the two
                denominator rows (at 32-aligned partitions, the only legal
                DVE write bases) into the batched den tile; frees the avs
                banks without waiting on any reciprocal."""
                with nc.allow_low_precision(reason="f32r keeps fp32 bits"):
                    for a in range(2):
                        nc.vector.tensor_scalar_mul(
                            dst_fn(f, a), avs[a][0:64, :], 1.0)
                        r = 32 * (2 * (f % 2) + a)
                        nc.vector.tensor_scalar_mul(
                            den[r:r + 1, :], avs[a][64:65, :], 1.0)

            def attn_norm2(fbase, den, upool, ps_b, dst_fn):
                """One batched fast-reciprocal for 2 f's (4 denom rows at
                partitions 0/32/64/96; the other 124 rows are memset to 1.0
                so their reciprocals stay finite), then a selector-matmul
                broadcast and in-place normalize."""
                recb = upool.tile([128, 512], f32r, name="recb")
                with nc.allow_low_precision(reason="f32r keeps fp32 bits"):
                    nc.vector.reciprocal(recb[:], den[:])
                for i in range(2):
                    f = fbase + i
                    bp = ps_b.tile([128, 512], f32, name="bp")
                    nc.tensor.matmul(bp[:], lhsT=esel[:, i * 128:(i + 1) * 128],
                                     rhs=recb[:], start=True, stop=True)
                    with nc.allow_low_precision(reason="f32r keeps fp32 bits"):
                        for a in range(2):
                            dst = dst_fn(f, a)
                            nc.vector.tensor_mul(
                                dst, dst, bp[64 * a:64 * (a + 1), :])

            def ln_stats_chunk(x_sb, mi, pm, pv, sqp):
                """Per-chunk LN statistic accumulation: mean partial-sum
                matmul + Pool square + var partial-sum matmul. Emitted right
                after chunk mi of x_sb is produced so it rides the host gemm's
                PE stream."""
                nc.tensor.matmul(pm, lhsT=onesp, rhs=x_sb[:, mi, :],
                                 start=(mi == 0), stop=(mi == 7))
                sq = sqp.tile([128, 512], f32r, name="sq")
                with nc.allow_low_precision(reason="f32r keeps fp32 bits"):
                    nc.gpsimd.tensor_mul(sq[:], x_sb[:, mi, :], x_sb[:, mi, :])
                nc.tensor.matmul(pv, lhsT=onesp, rhs=sq[:],
                                 start=(mi == 0), stop=(mi == 7))

            def ln_finish(x_sb, gC, bC, out_sb, ltp, pm, pv, binv, bmv,
                          out_dma=None):
                """LN tail given accumulated mean/var partial sums: scalar
                stats chain, inv-std via fast reciprocal + ACT sqrt, esel-free
                ones-broadcast matmuls, then per-chunk normalize split across
                DVE (muls) and Pool (scale+bias)."""
                msb = ltp.tile([1, 512], f32, name="lmsb")
                nc.vector.tensor_scalar_mul(msb[:], pm, 1.0 / D)
                sc = ltp.tile([1, 512], f32, name="lsc")
                sc2 = ltp.tile([1, 512], f32, name="lsc2")
                inv = ltp.tile([1, 512], f32r, name="linv")
                minv = ltp.tile([1, 512], f32r, name="lminv")
                nc.vector.tensor_scalar_mul(sc[:], pv, 1.0 / D)      # E[x^2]
                nc.gpsimd.tensor_mul(sc2[:], msb[:], msb[:])         # m^2
                nc.gpsimd.tensor_scalar_add(sc2[:], sc2[:], -EPS)
                nc.vector.tensor_tensor(out=sc[:], in0=sc[:], in1=sc2[:],
                                        op=ALU.subtract)             # var + eps
                nc.scalar.activation(sc2[:], sc[:], AF.Sqrt)         # sqrt(var+eps)
                with nc.allow_low_precision(reason="f32r keeps fp32 bits"):
                    nc.vector.reciprocal(inv[:], sc2[:])             # inv-std
                with nc.allow_low_precision(reason="f32r keeps fp32 bits"):
                    nc.vector.tensor_mul(minv[:], msb[:], inv[:])
                nc.tensor.matmul(binv, lhsT=ones1, rhs=inv[:],
                                 start=True, stop=True)
                nc.tensor.matmul(bmv, lhsT=ones1, rhs=minv[:], start=True, stop=True)
                with nc.allow_low_precision(reason="f32r keeps fp32 bits"):
                    for kc in range(8):
                        nc.vector.tensor_tensor(out=x_sb[:, kc, :], in0=x_sb[:, kc, :],
                                                in1=binv, op=ALU.mult)
                        nc.vector.tensor_tensor(out=x_sb[:, kc, :], in0=x_sb[:, kc, :],
                                                in1=bmv, op=ALU.subtract)
                        nc.gpsimd.tensor_scalar(
                            out=out_sb[:, kc, :], in0=x_sb[:, kc, :],
                            scalar1=gC[:, kc:kc + 1], scalar2=bC[:, kc:kc + 1],
                            op0=ALU.mult, op1=ALU.add)
                        if out_dma is not None:
                            out_dma(kc)

            # ================= phase 1: self projections =================
            with tc.tile_pool(name="ps1", bufs=8, space="PSUM") as ps:
                wp = wall

                def drain_q1(mi, n, pa):
                    nc.scalar.activation(qbuf[:, mi, :], pa[:], AF.Identity,
                                         bias=bcs["bq1c"][:, mi:mi + 1])
                gemm_TN(w_d["wq1"], lambda kc, n: dc_own[:, kc, :], 8, 8, 1,
                        drain_q1, wp, ps, dma=nc.sync.dma_start)

                def drain_k1(mi, n, pa):
                    nc.scalar.activation(kTv[:, mi, n * 512:(n + 1) * 512], pa[:],
                                         AF.Identity, bias=bcs["bk1c"][:, mi:mi + 1])
                gemm_TN(w_d["wk1"], lambda kc, n: dke[:, kc, n * 512:(n + 1) * 512],
                        8, 8, 2, drain_k1, wp, ps, dma=nc.sync.dma_start,
                        dma2=nc.scalar.dma_start)

                def drain_v1(rc, nf, pa):
                    dst = vaug[:, rc, nf * 8:(nf + 1) * 8, 0:64]
                    src = pa[:].rearrange("p (h d) -> p h d", h=8)
                    nc.scalar.activation(dst, src, AF.Copy, scale=vm1[:, rc:rc + 1])
                gemm_NT(w_d["wq1"], dke, 8, 8, 2, drain_v1, wp, ps, G=8,
                        dma=nc.scalar.dma_start, dma2=nc.sync.dma_start)
                for rc in range(8):
                    nc.sync.dma_start(out=vaug[:, rc, :, 64:65],
                                      in_=vm1r_d[:, rc, :].bitcast(f32r))

            # enc_output arrives in the (now-dead) dke slots; per-chunk DMAs
            # on the gpsimd queue so WAR waits don't head-of-line block the
            # weight streams.
            enc_r = enc_d.rearrange("(f p) r -> p f r", p=128).bitcast(f32r)
            for kc in range(8):
                nc.gpsimd.dma_start(out=encv[:, kc, :], in_=enc_r[:, kc, :])

            # ====== phase 2: self attention ++ q2/k2 cross projections ======
            with tc.tile_pool(name="ep2", bufs=4) as epool, \
                 tc.tile_pool(name="up2", bufs=2) as upool, \
                 tc.tile_pool(name="pss", bufs=2, space="PSUM") as ps_s, \
                 tc.tile_pool(name="psav", bufs=2, space="PSUM") as ps_av, \
                 tc.tile_pool(name="psb", bufs=1, space="PSUM") as ps_b, \
                 tc.tile_pool(name="psq", bufs=1, space="PSUM") as ps_q:
                dstA = lambda f, a: abufA[64 * a:64 * (a + 1), f, :]
                den = None
                for f in range(8):
                    if f % 2 == 0:
                        den = upool.tile([128, 512], f32, name="den")
                        nc.vector.memset(den[:], 1.0)

                    # q2 chunk f accumulates 1 matmul per chunk-slot into its
                    # own psum bank; slice f of qbuf is freed by this f's
                    # scores, so the drain lands there.
                    q2p = ps_q.tile([128, 512], f32, name="pq")

                    def fill_j(j, mi=f, pt=q2p):
                        wt = wall.tile([128, 128], f32r, name="wt")
                        nc.sync.dma_start(
                            out=wt,
                            in_=w_d["wq2"][j * 128:(j + 1) * 128,
                                           mi * 128:(mi + 1) * 128].bitcast(f32r))
                        nc.tensor.matmul(pt[:], lhsT=wt[:],
                                         rhs=dc_own[:, j, :],
                                         start=(j == 0), stop=(j == 7))

                    # k2T chunks lag one slot (slot f emits chunk f-1, slot 7
                    # emits 6 and 7) as slot-end bursts borrowing the ss
                    # rotation; they also cover the last exp of the slot.
                    def fill_end(mi=f, pt=q2p):
                        k2list = ([mi - 1] if 1 <= mi < 7
                                  else ([6, 7] if mi == 7 else []))
                        for m2 in k2list:
                            kp = ps_s.tile([128, 1024], f32, name="ss")
                            for kc in range(8):
                                wt = wall.tile([128, 128], f32r, name="wt")
                                nc.sync.dma_start(
                                    out=wt,
                                    in_=w_d["wk2"][kc * 128:(kc + 1) * 128,
                                                   m2 * 128:(m2 + 1) * 128].bitcast(f32r))
                                for n in range(2):
                                    nc.tensor.matmul(
                                        kp[:, n * 512:(n + 1) * 512],
                                        lhsT=wt[:],
                                        rhs=encv[:, kc, n * 512:(n + 1) * 512],
                                        start=(kc == 0), stop=(kc == 7))
                            for n in range(2):
                                nc.scalar.activation(
                                    kTv[:, m2, n * 512:(n + 1) * 512],
                                    kp[:, n * 512:(n + 1) * 512],
                                    AF.Identity, bias=bcs["bk2c"][:, m2:m2 + 1])

                    avs = attn_slot(f, qbuf, kTv, vaug, True, epool,
                                    ps_s, ps_av, fill_j, fill_end)
                    nc.scalar.activation(qbuf[:, f, :], q2p[:], AF.Identity,
                                         bias=bcs["bq2c"][:, f:f + 1])
                    attn_stash(f, avs, den, dstA)
                    if f % 2 == 1:
                        attn_norm2(f - 1, den, upool, ps_b, dstA)
            mk_ctx.__exit__(None, None, None)

            # ================= phase 3: v2 cross projection =================
            with tc.tile_pool(name="ps3", bufs=8, space="PSUM") as ps:
                def drain_v2(rc, nf, pa):
                    dst = vaug[:, rc, nf * 8:(nf + 1) * 8, 0:64]
                    src = pa[:].rearrange("p (h d) -> p h d", h=8)
                    nc.scalar.activation(dst, src, AF.Copy, scale=vm2[:, rc:rc + 1])
                gemm_NT(w_d["wq2"], encv, 8, 8, 2, drain_v2, wall, ps, G=8,
                        dma=nc.scalar.dma_start, dma2=nc.sync.dma_start)
                for rc in range(8):
                    nc.sync.dma_start(out=vaug[:, rc, :, 64:65],
                                      in_=vm2r_d[:, rc, :].bitcast(f32r))

            # ============ phase 4: cross attention ++ wo1 ++ LN1 ============
            with tc.tile_pool(name="ep4", bufs=3) as epool, \
                 tc.tile_pool(name="up4", bufs=2) as upool, \
                 tc.tile_pool(name="sq4", bufs=1) as sqp4, \
                 tc.tile_pool(name="lt4", bufs=1) as ltp4, \
                 tc.tile_pool(name="pss4", bufs=2, space="PSUM") as ps_s, \
                 tc.tile_pool(name="psav4", bufs=2, space="PSUM") as ps_av, \
                 tc.tile_pool(name="psb4", bufs=1, space="PSUM") as ps_b, \
                 tc.tile_pool(name="psq4", bufs=1, space="PSUM") as ps_q:
                # attn2 output lands in qbuf in-place: slice f of q2T is dead
                # once this f's score matmuls have read it.
                dstB = lambda f, a: qbuf[64 * a:64 * (a + 1), f, :]
                den = None
                ln1_pm = ln1_pv = None
                for f in range(8):
                    if f % 2 == 0:
                        den = upool.tile([128, 512], f32, name="den")
                        nc.vector.memset(den[:], 1.0)

                    # wo1 chunks 2f and 2f+1 interleave into slots 0-3 (they
                    # only depend on attn1/dec, both long ready): chunk 2f
                    # accumulates over chunk-slots 0-3, chunk 2f+1 over 4-7,
                    # so one psum bank rotates between them.
                    state = {"pt": None}

                    def fill_j(j, mi_base=f):
                        if mi_base >= 4:
                            return
                        mi = 2 * mi_base + (j // 4)
                        if j % 4 == 0:
                            state["pt"] = ps_q.tile([128, 512], f32, name="pq")
                        pt = state["pt"]
                        for kc in (2 * (j % 4), 2 * (j % 4) + 1):
                            wt = wall.tile([128, 128], f32r, name="wt")
                            nc.sync.dma_start(
                                out=wt,
                                in_=w_d["wo1"][kc * 128:(kc + 1) * 128,
                                               mi * 128:(mi + 1) * 128].bitcast(f32r))
                            nc.tensor.matmul(pt[:], lhsT=wt[:],
                                             rhs=abufA[:, kc, :],
                                             start=(kc == 0), stop=(kc == 7))
                        if j % 4 == 3:
                            # fused drain: xa = (psum + bo1) + dec residual
                            with nc.allow_low_precision(reason="f32r keeps fp32 bits"):
                                nc.vector.scalar_tensor_tensor(
                                    out=xa[:, mi, :], in0=pt[:],
                                    scalar=bcs["bo1c"][:, mi:mi + 1],
                                    in1=dc_own[:, mi, :],
                                    op0=ALU.add, op1=ALU.add)

                    def fill_end(mi_base=f):
                        pass

                    avs = attn_slot(f, qbuf, kTv, vaug, False, epool,
                                    ps_s, ps_av, fill_j, fill_end)
                    attn_stash(f, avs, den, dstB)
                    if f % 2 == 1:
                        attn_norm2(f - 1, den, upool, ps_b, dstB)

                    if f == 3:
                        # LN1 stats burst on xa: the matmuls + squares fill
                        # the otherwise-empty slots 4-7; normalize chunks run
                        # on DVE/Pool (ACT is busy with cross exps). Mean
                        # rides the freed wo1 bank, var the norm2 bank.
                        ln1_pmt = ps_q.tile([128, 512], f32, name="pq")
                        ln1_pvt = ps_b.tile([128, 512], f32, name="bp")
                        ln1_pm = ln1_pmt[0:1, :]
                        ln1_pv = ln1_pvt[0:1, :]
                        for mi in range(8):
                            nc.tensor.matmul(ln1_pm, lhsT=onesp,
                                             rhs=xa[:, mi, :],
                                             start=(mi == 0), stop=(mi == 7))
                        for mi in range(8):
                            sq = sqp4.tile([128, 512], f32r, name="sq")
                            with nc.allow_low_precision(reason="f32r keeps fp32 bits"):
                                nc.gpsimd.tensor_mul(sq[:], xa[:, mi, :],
                                                     xa[:, mi, :])
                            nc.tensor.matmul(ln1_pv, lhsT=onesp,
                                             rhs=sq[:],
                                             start=(mi == 0), stop=(mi == 7))
                    if f == 4:
                        ln1_binv = ps_q.tile([128, 512], f32, name="pq")
                        ln1_bmv = ps_b.tile([128, 512], f32, name="bp")
                        ln_finish(xa, bcs["g1c"], bcs["b1c"], xa, ltp4,
                                  ln1_pm, ln1_pv,
                                  ln1_binv[:], ln1_bmv[:])

            # ============ phase 5: wo2 + folded LN2 ============
            with tc.tile_pool(name="sq5", bufs=2) as sqp, \
                 tc.tile_pool(name="lt5", bufs=1) as ltp, \
                 tc.tile_pool(name="ps5", bufs=4, space="PSUM") as ps, \
                 tc.tile_pool(name="ps5s", bufs=2, space="PSUM") as ps_s5:
                ln2_pm = ps_s5.tile([1, 512], f32, name="pm")
                ln2_pv = ps_s5.tile([1, 512], f32, name="pv")

                def drain_wo2(mi, n, pa):
                    with nc.allow_low_precision(reason="f32r keeps fp32 bits"):
                        nc.vector.scalar_tensor_tensor(
                            out=abufA[:, mi, :], in0=pa[:],
                            scalar=bcs["bo2c"][:, mi:mi + 1],
                            in1=xa[:, mi, :], op0=ALU.add, op1=ALU.add)
                    ln_stats_chunk(abufA, mi, ln2_pm[:], ln2_pv[:], sqp)
                gemm_TN(w_d["wo2"], lambda kc, n: qbuf[:, kc, :], 8, 8, 1,
                        drain_wo2, wall, ps, dma=nc.sync.dma_start,
                        dma2=nc.scalar.dma_start)

                ln2_binv = ps.tile([128, 512], f32, name="pp")
                ln2_bmv = ps.tile([128, 512], f32, name="pp")
                ln_finish(abufA, bcs["g2c"], bcs["b2c"], abufA, ltp,
                          ln2_pm[:], ln2_pv[:], ln2_binv[:], ln2_bmv[:])

            # ============ phase 6: FFN + folded LN3 + output =============
            with tc.tile_pool(name="sq6", bufs=2) as sqp, \
                 tc.tile_pool(name="lt6", bufs=1) as ltp, \
                 tc.tile_pool(name="ps6", bufs=4, space="PSUM") as ps, \
                 tc.tile_pool(name="ps6b", bufs=1, space="PSUM") as ps8:
                wp = wall

                def drain_f1(mi, n, pa):
                    nc.scalar.activation(arena[:, mi * 512:(mi + 1) * 512], pa[:],
                                         AF.Relu, bias=fb1c[:, mi:mi + 1])
                gemm_TN(fw1_d, lambda kc, n: abufA[:, kc, :], 8, 32, 1,
                        drain_f1, wp, ps, dma=nc.sync.dma_start,
                        dma2=nc.scalar.dma_start)

                ln3_pm = ps.tile([128, 512], f32, name="pp")
                ln3_pv = ps.tile([128, 512], f32, name="pp")

                # ffn2 in two 4-bank halves so it shares PSUM with ffn1 and
                # its matmuls can fill ffn1's weight-DMA gaps; LN3 statistics
                # ride the drain stream.
                for mh in range(2):
                    pps = [ps8.tile([128, 512], f32, name=f"pf_{i}")
                           for i in range(4)]
                    for kc in range(32):
                        wt = wp.tile([128, 512], f32r, name="wt")
                        dq = nc.scalar.dma_start if kc % 2 == 0 else nc.sync.dma_start
                        dq(out=wt,
                           in_=fw2_d[kc * 128:(kc + 1) * 128,
                                     mh * 512:(mh + 1) * 512].bitcast(f32r))
                        for i in range(4):
                            nc.tensor.matmul(
                                pps[i][:],
                                lhsT=wt[:, i * 128:(i + 1) * 128],
                                rhs=arena[:, kc * 512:(kc + 1) * 512],
                                start=(kc == 0), stop=(kc == 31))
                    for i in range(4):
                        mi = mh * 4 + i
                        with nc.allow_low_precision(reason="f32r keeps fp32 bits"):
                            nc.vector.scalar_tensor_tensor(
                                out=xa[:, mi, :], in0=pps[i][:],
                                scalar=bcs["fb2c"][:, mi:mi + 1],
                                in1=abufA[:, mi, :], op0=ALU.add, op1=ALU.add)
                        ln_stats_chunk(xa, mi, ln3_pm[0:1, :], ln3_pv[0:1, :],
                                       sqp)

                def out_dma(kc):
                    nc.sync.dma_start(
                        out=out_d[kc * 128:(kc + 1) * 128, :].bitcast(f32r),
                        in_=qbuf[:, kc, :])
                ln3_binv = ps.tile([128, 512], f32, name="pp")
                ln3_bmv = ps.tile([128, 512], f32, name="pp")
                ln_finish(xa, bcs["g3c"], bcs["b3c"], qbuf, ltp,
                          ln3_pm[0:1, :], ln3_pv[0:1, :],
                          ln3_binv[:], ln3_bmv[:], out_dma=out_dma)
            ctx_wp.__exit__(None, None, None)

    _split_waits(nc, 1)
    return nc


_PROGRAM = None


def _get_program():
    global _PROGRAM
    if _PROGRAM is None:
        _PROGRAM = build_program()
    return _PROGRAM


def _esel_data():
    """esel[p, i*128 + m] = 1 iff p == 32*(2i + (m >= 64)): selector so that
    esel[:, i*128:(i+1)*128]^T @ recb broadcasts recb row 32*2i to output
    partitions 0:64 and row 32*(2i+1) to partitions 64:128."""
    e = np.zeros((128, 256), np.float32)
    for i in range(2):
        e[32 * 2 * i, i * 128:i * 128 + 64] = 1.0
        e[32 * (2 * i + 1), i * 128 + 64:(i + 1) * 128] = 1.0
    return e


def _core_inputs(inp, c):
    b, j = c // 2, c % 2
    dec = np.asarray(inp["dec_input"][b], np.float32)      # [S, D]
    enc = np.asarray(inp["enc_output"][b], np.float32)
    decT = np.ascontiguousarray(dec.T)                     # [D, S]
    own = np.ascontiguousarray(decT[:, j * R:(j + 1) * R])
    if j == 1:
        dke = decT                                         # ctx = rows 0:512, diag = 512:1024
    else:
        dke = np.ascontiguousarray(
            np.concatenate([decT[:, R:], decT[:, :R]], axis=1))
    la = np.asarray(inp["look_ahead_mask"], np.float32)[0, 0]
    maskT = np.ascontiguousarray(la[j * R:(j + 1) * R, j * R:(j + 1) * R].T) * np.float32(-8e9)
    padb = (np.asarray(inp["padding_mask"], np.float32)[b, 0, 0] * np.float32(-1e9))
    vm = np.ones(S, np.float32)
    if j == 0:
        vm[:R] = 0.0                                       # ctx block invalid for first half
    v2 = np.ones(S, np.float32)

    def chunk(a, n):
        return np.ascontiguousarray(np.asarray(a, np.float32).reshape(n, 128).T)

    wo1 = np.asarray(inp["wo1"], np.float32)
    wo2 = np.asarray(inp["wo2"], np.float32)
    bo1e = np.asarray(inp["bq1"], np.float32) @ wo1 + np.asarray(inp["bo1"], np.float32)
    bo2e = np.asarray(inp["bq2"], np.float32) @ wo2 + np.asarray(inp["bo2"], np.float32)

    return {
        "dc_own": own, "dke": dke,
        "encT": np.ascontiguousarray(enc.T),
        "maskT": maskT,
        "padb": chunk(padb, 8),
        "vm1": chunk(vm, 8),
        "vm1r": np.repeat(chunk(vm, 8)[:, :, None], 16, axis=2),
        "vm2": chunk(v2, 8),
        "vm2r": np.ones((128, 8, 16), np.float32),
        "onesd": np.ones((128, 128), np.float32),
        "esel": _esel_data(),
        "wq1": np.asarray(inp["wq1"], np.float32),
        "wk1": np.asarray(inp["wk1"], np.float32),
        "wq2": np.asarray(inp["wq2"], np.float32),
        "wk2": np.asarray(inp["wk2"], np.float32),
        "wo1": wo1, "wo2": wo2,
        "fw1": np.asarray(inp["ff_w1"], np.float32),
        "fw2": np.asarray(inp["ff_w2"], np.float32),
        "bq1c": chunk(inp["bq1"], 8), "bk1c": chunk(inp["bk1"], 8),
        "bq2c": chunk(inp["bq2"], 8), "bk2c": chunk(inp["bk2"], 8),
        "bo1c": chunk(bo1e, 8), "bo2c": chunk(bo2e, 8),
        "fb1c": chunk(inp["ff_b1"], 32), "fb2c": chunk(inp["ff_b2"], 8),
        "g1c": chunk(inp["ln1_g"], 8), "b1c": chunk(inp["ln1_b"], 8),
        "g2c": chunk(inp["ln2_g"], 8), "b2c": chunk(inp["ln2_b"], 8),
        "g3c": chunk(inp["ln3_g"], 8), "b3c": chunk(inp["ln3_b"], 8),
    }


def kernel(**inputs):
    nc = _get_program()
    in_maps = [_core_inputs(inputs, c) for c in range(N_CORES)]
    res = run_bass_kernel_spmd(nc, in_maps, list(range(N_CORES)))
    out = np.empty((B, S, D), np.float32)
    for c in range(N_CORES):
        b, j = c // 2, c % 2
        out[b, j * R:(j + 1) * R, :] = res.results[c]["outT"].T
    return out


if __name__ == "__main__":
    import tempfile
    from concourse.bass_utils import compile_bass_kernel
    nc = build_program()
    with tempfile.TemporaryDirectory() as td:
        compile_bass_kernel(nc, td)
    print("COMPILE OK")
